# revision 1
# baseline (speedup 1.0000x reference)
"""LoRA LayerNorm Trainium2 kernel (8-core data-parallel, raw Bass).

out = x_hat * scale + shift, where
  x_hat    = (x - mean) * rsqrt(var + eps)        (LayerNorm over last dim)
  scale[i] = sum_r A_s[i,r] * B_s[r,i] * 2.0      (low-rank diagonal)
  shift[i] = sum_r A_h[i,r] * B_h[r,i] * 2.0

Sharding: x [2,4096,8192] -> 8192 rows, 1024 rows per core. LoRA params
replicated; each core computes scale/shift redundantly on device.

Per-core algorithm (rows on partitions, 8 tiles of [128, 8192]):
  setup: scale/shift diagonals via strided loads + DVE mul/reduce,
         bounced through DRAM to produce scale broadcast [128,8192] and a
         shift row [1,8192]; PSUM has_written bits pre-set by one
         start=True matmul per bank (values are overwritten later).
  per tile:
    DVE : bn_stats/bn_aggr -> mean,var; psum = (x - mean) * scale_bcast
    ACT : std = sqrt(var+eps); out_sbuf = psum * rstd  (PSUM->SBUF copy)
    PE  : psum += std (x) shift  (K=1 rank-1 accumulate, start=False)
    SP  : x tile loads (HWDGE);  ACT ring: output stores (HWDGE)
    POOL: tiny SBUF->SBUF DMA transposing std [128,1] -> stdT [1,128]
"""

import numpy as np
from contextlib import ExitStack

import concourse.bass as bass
from concourse import mybir
from concourse.bass_utils import run_bass_kernel_spmd

F32 = mybir.dt.float32

# Problem geometry (hardcoded; see module docstring)
B_DIM, S_DIM, N = 2, 4096, 8192
RANK = 4
SCALING = 2.0  # alpha / rank = 8 / 4
EPS = 1e-5
NCORES = 8
ROWS = B_DIM * S_DIM // NCORES  # 1024 rows per core
P = 128
NTILES = ROWS // P              # 8
CHUNK = 2048                    # psum chunk (4 banks)
NCHUNK = N // CHUNK             # 4
HALF = N // 2                   # output store granularity
BN_F = 512                      # bn_stats max free dim
NBN = N // BN_F                 # 16
NSL = CHUNK // 512              # matmul slices per chunk


def build_nc() -> bass.Bass:
    nc = bass.Bass()

    x = nc.declare_dram_parameter("x_shard", [ROWS, N], F32, isOutput=False)
    sa = nc.declare_dram_parameter("lora_scale_A", [N, RANK], F32, isOutput=False)
    sb = nc.declare_dram_parameter("lora_scale_B", [RANK, N], F32, isOutput=False)
    ha = nc.declare_dram_parameter("lora_shift_A", [N, RANK], F32, isOutput=False)
    hb = nc.declare_dram_parameter("lora_shift_B", [RANK, N], F32, isOutput=False)
    y = nc.declare_dram_parameter("y_shard", [ROWS, N], F32, isOutput=True)

    scale_vec = nc.dram_tensor("scale_vec", [N], F32)
    shift_vec = nc.dram_tensor("shift_vec", [N], F32)

    with ExitStack() as ctx:
        ec = ctx.enter_context
        # big tiles
        xb = [ec(nc.sbuf_tensor(f"xb{i}", [P, N], F32)) for i in range(2)]
        outb = [ec(nc.sbuf_tensor(f"outb{i}", [P, HALF], F32)) for i in range(2)]
        scale_bc = ec(nc.sbuf_tensor("scale_bc", [P, N], F32))
        sh_row = ec(nc.sbuf_tensor("sh_row", [1, N], F32))
        # setup scratch
        a_t = ec(nc.sbuf_tensor("a_t", [P, (N // P) * RANK], F32))  # [128, 256]
        b_t = ec(nc.sbuf_tensor("b_t", [P, RANK * (N // P)], F32))  # [128, 256]
        prod = ec(nc.sbuf_tensor("prod", [P, (N // P) * RANK], F32))
        s_small = ec(nc.sbuf_tensor("s_small", [P, N // P], F32))   # [128, 64]
        t_small = ec(nc.sbuf_tensor("t_small", [P, N // P], F32))
        # per-tile stats
        stats = ec(nc.sbuf_tensor("stats", [P, NBN * 6], F32))
        mv = ec(nc.sbuf_tensor("mv", [P, 2], F32))
        nm = ec(nc.sbuf_tensor("nm", [P, 1], F32))
        stdb = [ec(nc.sbuf_tensor(f"stdb{i}", [P, 1], F32)) for i in range(2)]
        rstdb = [ec(nc.sbuf_tensor(f"rstdb{i}", [P, 1], F32)) for i in range(2)]
        stdT = [ec(nc.sbuf_tensor(f"stdT{i}", [1, P], F32)) for i in range(2)]
        eps_t = ec(nc.sbuf_tensor("eps_t", [P, 1], F32))
        zrow = ec(nc.sbuf_tensor("zrow", [1, 512], F32))
        # psum
        pz = [ec(nc.psum_tensor(f"pz{i}", [P, CHUNK], F32)) for i in range(2)]

        sems = {}
        for s in ("load0", "load1", "store0", "store1", "stdT0", "stdT1",
                  "stt", "stats", "std", "rstd", "acc", "copy", "const",
                  "sdma", "dset", "gset", "pset"):
            sems[s] = ec(nc.semaphore(s))
        loadS = [sems["load0"], sems["load1"]]
        storeS = [sems["store0"], sems["store1"]]
        stdTS = [sems["stdT0"], sems["stdT1"]]

        C = N // P  # 64

        with nc.Block() as block:

            @block.sync
            def _(sp):
                for t in range(NTILES):
                    if t >= 2:
                        # x buffer t%2 free once DVE finished stt of tile t-2
                        sp.wait_ge(sems["stt"], NCHUNK * (t - 1))
                    sp.dma_start(
                        out=xb[t % 2][:], in_=x[t * P:(t + 1) * P, :]
                    ).then_inc(loadS[t % 2], 16)

            @block.gpsimd
            def _(gp):
                # setup: load scale pair (A as [p,(c r)], B as [p,(r c)])
                gp.dma_start(
                    out=a_t[:],
                    in_=sa[:, :].rearrange("(p c) r -> p (c r)", p=P),
                ).then_inc(sems["sdma"], 16)
                gp.dma_start(
                    out=b_t[:].rearrange("p (r c) -> p r c", r=RANK),
                    in_=sb[:, :].rearrange("r (p c) -> p r c", p=P),
                ).then_inc(sems["sdma"], 16)
                gp.wait_ge(sems["dset"], 1)
                gp.dma_start(
                    out=scale_vec[:].rearrange("(p c) -> p c", p=P),
                    in_=s_small[:],
                ).then_inc(sems["gset"], 16)
                # reuse a_t/b_t for the shift pair
                gp.dma_start(
                    out=a_t[:],
                    in_=ha[:, :].rearrange("(p c) r -> p (c r)", p=P),
                ).then_inc(sems["sdma"], 16)
                gp.dma_start(
                    out=b_t[:].rearrange("p (r c) -> p r c", r=RANK),
                    in_=hb[:, :].rearrange("r (p c) -> p r c", p=P),
                ).then_inc(sems["sdma"], 16)
                gp.wait_ge(sems["dset"], 2)
                gp.dma_start(
                    out=shift_vec[:].rearrange("(p c) -> p c", p=P),
                    in_=t_small[:],
                ).then_inc(sems["gset"], 16)
                # both DRAM vectors written before reading them back
                gp.wait_ge(sems["gset"], 32)
                # broadcast scale along partitions (stride-0 DRAM read)
                sv = scale_vec[:]
                gp.dma_start(
                    out=scale_bc[:],
                    in_=bass.AP(tensor=sv.tensor, offset=sv.offset,
                                ap=[[0, P]] + list(sv.ap)),
                ).then_inc(sems["gset"], 16)
                gp.dma_start(out=sh_row[:], in_=shift_vec[:]).then_inc(
                    sems["gset"], 16
                )
                # per-tile: transpose std [128,1] -> stdT [1,128]
                for t in range(NTILES):
                    gp.wait_ge(sems["std"], t + 1)
                    if t >= 2:
                        # PE done reading stdT[t%2] (accums of tile t-2)
                        gp.wait_ge(sems["acc"], NCHUNK * NSL * (t - 1))
                    gp.dma_start(
                        out=stdT[t % 2][:], in_=stdb[t % 2][:]
                    ).then_inc(stdTS[t % 2], 16)

            @block.vector
            def _(v):
                v.memset(eps_t[:], EPS).then_inc(sems["const"], 1)
                v.memset(zrow[:], 0.0).then_inc(sems["const"], 1)
                # low-rank diagonals: diag = sum_r A[:,r]*B[r,:] * SCALING
                for (small, k) in ((s_small, 1), (t_small, 2)):
                    v.wait_ge(sems["sdma"], 32 * k)
                    v.tensor_mul(
                        prod[:].rearrange("p (c r) -> p c r", c=C),
                        a_t[:].rearrange("p (c r) -> p c r", c=C),
                        b_t[:].rearrange("p (r c) -> p c r", r=RANK),
                    )
                    v.drain()
                    v.tensor_reduce(
                        out=small[:].rearrange("p (c u) -> p c u", u=1),
                        in_=prod[:].rearrange("p (c r) -> p c r", c=C),
                        axis=mybir.AxisListType.X,
                        op=mybir.AluOpType.add,
                    )
                    v.drain()
                    v.tensor_scalar_mul(small[:], small[:], SCALING).then_inc(
                        sems["dset"], 1
                    )
                v.wait_ge(sems["gset"], 64)  # scale_bc + sh_row resident
                v.wait_ge(sems["pset"], 2 * NSL)  # PSUM bits pre-set by PE
                for t in range(NTILES):
                    v.wait_ge(loadS[t % 2], 16 * (t // 2 + 1))
                    xt = xb[t % 2]
                    for c in range(NBN):
                        v.bn_stats(
                            out=stats[:].rearrange("p (c s) -> p c s", s=6)[
                                :, c, :
                            ],
                            in_=xt[:, c * BN_F:(c + 1) * BN_F],
                        )
                    v.drain()
                    v.bn_aggr(
                        out=mv[:],
                        in_=stats[:].rearrange("p (c s) -> p c s", s=6),
                    ).then_inc(sems["stats"], 1)
                    v.drain()
                    v.tensor_scalar_mul(nm[:], mv[:, 0:1], -1.0)
                    v.drain()
                    if t >= 2:
                        # rstd buffer free (ACT copies of tile t-2 done)
                        v.wait_ge(sems["copy"], NCHUNK * (t - 1))
                    v.wait_ge(sems["std"], t + 1)
                    v.reciprocal(rstdb[t % 2][:], stdb[t % 2][:]).then_inc(
                        sems["rstd"], 1
                    )
                    for c in range(NCHUNK):
                        g = NCHUNK * t + c
                        if g >= 2:
                            # psum buffer g%2 free (ACT copied chunk g-2)
                            v.wait_ge(sems["copy"], g - 1)
                        v.scalar_tensor_tensor(
                            out=pz[g % 2][:],
                            in0=xt[:, c * CHUNK:(c + 1) * CHUNK],
                            scalar=nm[:],
                            in1=scale_bc[:, c * CHUNK:(c + 1) * CHUNK],
                            op0=mybir.AluOpType.add,
                            op1=mybir.AluOpType.mult,
                        ).then_inc(sems["stt"], 1)

            @block.tensor
            def _(te):
                # pre-set PSUM has_written bits once per bank: a start=True
                # matmul writing zeros. Values are overwritten by DVE each
                # chunk; later start=False matmuls then accumulate.
                te.wait_ge(sems["const"], 2)
                for b in range(2):
                    for s in range(NSL):
                        nc.tensor.matmul(
                            pz[b][:, s * 512:(s + 1) * 512],
                            zrow[:, 0:P],
                            zrow[:, 0:512],
                            start=True,
                            stop=True,
                        ).then_inc(sems["pset"], 1)
                for t in range(NTILES):
                    te.wait_ge(stdTS[t % 2], 16 * (t // 2 + 1))
                    for c in range(NCHUNK):
                        g = NCHUNK * t + c
                        te.wait_ge(sems["stt"], g + 1)
                        for s in range(NSL):
                            j = c * CHUNK + s * 512
                            nc.tensor.matmul(
                                pz[g % 2][:, s * 512:(s + 1) * 512],
                                stdT[t % 2][:],
                                sh_row[:, j:j + 512],
                                start=False,
                                stop=True,
                                skip_group_check=True,
                            ).then_inc(sems["acc"], 1)

            @block.scalar
            def _(sc):
                sc.wait_ge(sems["const"], 1)  # eps
                for t in range(NTILES):
                    sc.wait_ge(sems["stats"], t + 1)
                    if t >= 2:
                        # std buffer free (gpsimd copied std of tile t-2)
                        sc.wait_ge(stdTS[t % 2], 16 * (t // 2))
                    sc.activation(
                        out=stdb[t % 2][:],
                        in_=mv[:, 1:2],
                        func=mybir.ActivationFunctionType.Sqrt,
                        bias=eps_t[:],
                        scale=1.0,
                    ).then_inc(sems["std"], 1)
                    sc.wait_ge(sems["rstd"], t + 1)
                    for c in range(NCHUNK):
                        g = NCHUNK * t + c
                        h = c // 2
                        off = (c % 2) * CHUNK
                        sc.wait_ge(sems["acc"], NSL * (g + 1))
                        if c % 2 == 0 and t >= 1:
                            # out buffer h free (store of tile t-1 done)
                            sc.wait_ge(storeS[h], 16 * t)
                        sc.activation(
                            out=outb[h][:, off:off + CHUNK],
                            in_=pz[g % 2][:],
                            func=mybir.ActivationFunctionType.Copy,
                            bias=0.0,
                            scale=rstdb[t % 2][:],
                        ).then_inc(sems["copy"], 1)
                        if c % 2 == 1:
                            sc.drain()
                            sc.dma_start(
                                out=y[t * P:(t + 1) * P,
                                      h * HALF:(h + 1) * HALF],
                                in_=outb[h][:],
                            ).then_inc(storeS[h], 16)

    return nc


def kernel(x, lora_scale_A, lora_scale_B, lora_shift_A, lora_shift_B):
    x = np.ascontiguousarray(np.asarray(x, dtype=np.float32).reshape(-1, N))
    args = {
        "lora_scale_A": np.ascontiguousarray(lora_scale_A, dtype=np.float32),
        "lora_scale_B": np.ascontiguousarray(lora_scale_B, dtype=np.float32),
        "lora_shift_A": np.ascontiguousarray(lora_shift_A, dtype=np.float32),
        "lora_shift_B": np.ascontiguousarray(lora_shift_B, dtype=np.float32),
    }
    in_maps = [
        {"x_shard": x[i * ROWS:(i + 1) * ROWS], **args} for i in range(NCORES)
    ]
    nc = build_nc()
    res = run_bass_kernel_spmd(nc, in_maps, core_ids=list(range(NCORES)))
    out = np.concatenate(
        [res.results[i]["y_shard"] for i in range(NCORES)], axis=0
    )
    return out.reshape(B_DIM, S_DIM, N)


if __name__ == "__main__":
    import reference

    inputs = {k: np.asarray(v) for k, v in reference.setup_inputs().items()}
    expected = np.asarray(reference.reference(**inputs))
    actual = kernel(**inputs)
    err = np.abs(actual - expected)
    denom = np.abs(expected).max()
    print("max abs err:", err.max(), "rel:", err.max() / denom)



# revision 2
# speedup vs baseline: 1.7282x; 1.7282x over previous
"""LoRA LayerNorm Trainium2 kernel (8-core data-parallel, raw Bass).

out = x_hat * scale + shift, where
  x_hat    = (x - mean) * rsqrt(var + eps)        (LayerNorm over last dim)
  scale[i] = sum_r A_s[i,r] * B_s[r,i] * 2.0      (low-rank diagonal)
  shift[i] = sum_r A_h[i,r] * B_h[r,i] * 2.0

Sharding: x [2,4096,8192] -> 8192 rows, 1024 rows per core. LoRA params
replicated; each core computes scale/shift redundantly on device.

The output is computed and stored in float16 (tolerance is 2e-2; the f16
pipeline lands ~8e-4) which halves the store-side HBM traffic. The host
converts back to f32.

Per-core algorithm (rows on partitions, 8 tiles of [128, 8192]):
  setup: scale/shift diagonals via strided loads + DVE mul/reduce, cast to
         f16 and bounced through DRAM to produce broadcast tiles
         scale_bc/shift_bc [128, 8192] f16.
  per tile:
    SP  : x tile load (HWDGE, f32)
    DVE : bn_stats/bn_aggr -> mean,var; reciprocal -> rstd; nmr = -mean*rstd
    ACT : std = sqrt(var+eps); xh = Identity(x*rstd + nmr)  f32 -> f16
    DVE : xh *= scale_bc ; xh += shift_bc   (f16 tensor_tensor, 2x rate)
    ACT : store xh -> y (HWDGE, f16)
"""

import numpy as np
from contextlib import ExitStack

import concourse.bass as bass
from concourse import mybir
from concourse.bass_utils import run_bass_kernel_spmd

F32 = mybir.dt.float32
F16 = mybir.dt.float16

# Problem geometry (hardcoded; see module docstring)
B_DIM, S_DIM, N = 2, 4096, 8192
RANK = 4
SCALING = 2.0  # alpha / rank = 8 / 4
EPS = 1e-5
NCORES = 8
ROWS = B_DIM * S_DIM // NCORES  # 1024 rows per core
P = 128
NTILES = ROWS // P              # 8
BN_F = 512                      # bn_stats max free dim
NBN = N // BN_F                 # 16


def build_nc() -> bass.Bass:
    nc = bass.Bass()

    x = nc.declare_dram_parameter("x_shard", [ROWS, N], F32, isOutput=False)
    sa = nc.declare_dram_parameter("lora_scale_A", [N, RANK], F32, isOutput=False)
    sb = nc.declare_dram_parameter("lora_scale_B", [RANK, N], F32, isOutput=False)
    ha = nc.declare_dram_parameter("lora_shift_A", [N, RANK], F32, isOutput=False)
    hb = nc.declare_dram_parameter("lora_shift_B", [RANK, N], F32, isOutput=False)
    y = nc.declare_dram_parameter("y_shard", [ROWS, N], F16, isOutput=True)

    scale_vec = nc.dram_tensor("scale_vec", [N], F16)
    shift_vec = nc.dram_tensor("shift_vec", [N], F16)

    with ExitStack() as ctx:
        ec = ctx.enter_context
        # big tiles
        xb = [ec(nc.sbuf_tensor(f"xb{i}", [P, N], F32)) for i in range(2)]
        xh = [ec(nc.sbuf_tensor(f"xh{i}", [P, N], F16)) for i in range(2)]
        scale_bc = ec(nc.sbuf_tensor("scale_bc", [P, N], F16))
        shift_bc = ec(nc.sbuf_tensor("shift_bc", [P, N], F16))
        # setup scratch
        a_t = ec(nc.sbuf_tensor("a_t", [P, (N // P) * RANK], F32))  # [128, 256]
        b_t = ec(nc.sbuf_tensor("b_t", [P, RANK * (N // P)], F32))  # [128, 256]
        prod = ec(nc.sbuf_tensor("prod", [P, (N // P) * RANK], F32))
        s_small = ec(nc.sbuf_tensor("s_small", [P, N // P], F32))   # [128, 64]
        t_small = ec(nc.sbuf_tensor("t_small", [P, N // P], F32))
        sv16 = ec(nc.sbuf_tensor("sv16", [P, N // P], F16))
        tv16 = ec(nc.sbuf_tensor("tv16", [P, N // P], F16))
        # per-tile stats
        stats = ec(nc.sbuf_tensor("stats", [P, NBN * 6], F32))
        mv = [ec(nc.sbuf_tensor(f"mv{i}", [P, 2], F32)) for i in range(2)]
        stdb = [ec(nc.sbuf_tensor(f"stdb{i}", [P, 1], F32)) for i in range(2)]
        rstdb = [ec(nc.sbuf_tensor(f"rstdb{i}", [P, 1], F32)) for i in range(2)]
        nmrb = [ec(nc.sbuf_tensor(f"nmrb{i}", [P, 1], F32)) for i in range(2)]
        eps_t = ec(nc.sbuf_tensor("eps_t", [P, 1], F32))

        sems = {}
        for s in ("load0", "load1", "store0", "store1", "sdma", "dset",
                  "gset", "stats", "std", "nmr", "xh", "tt", "const"):
            sems[s] = ec(nc.semaphore(s))
        loadS = [sems["load0"], sems["load1"]]
        storeS = [sems["store0"], sems["store1"]]

        C = N // P  # 64
        MUL = mybir.AluOpType.mult

        with nc.Block() as block:

            @block.sync
            def _(sp):
                for t in range(NTILES):
                    if t >= 2:
                        # xb[t%2] free once ACT finished xh of tile t-2
                        sp.wait_ge(sems["xh"], t - 1)
                    sp.dma_start(
                        out=xb[t % 2][:], in_=x[t * P:(t + 1) * P, :]
                    ).then_inc(loadS[t % 2], 16)

            @block.gpsimd
            def _(gp):
                # setup: load scale pair (A as [p,(c r)], B as [p,(r c)])
                gp.dma_start(
                    out=a_t[:],
                    in_=sa[:, :].rearrange("(p c) r -> p (c r)", p=P),
                ).then_inc(sems["sdma"], 16)
                gp.dma_start(
                    out=b_t[:].rearrange("p (r c) -> p r c", r=RANK),
                    in_=sb[:, :].rearrange("r (p c) -> p r c", p=P),
                ).then_inc(sems["sdma"], 16)
                gp.wait_ge(sems["dset"], 1)
                gp.dma_start(
                    out=scale_vec[:].rearrange("(p c) -> p c", p=P),
                    in_=sv16[:],
                ).then_inc(sems["gset"], 16)
                # reuse a_t/b_t for the shift pair
                gp.dma_start(
                    out=a_t[:],
                    in_=ha[:, :].rearrange("(p c) r -> p (c r)", p=P),
                ).then_inc(sems["sdma"], 16)
                gp.dma_start(
                    out=b_t[:].rearrange("p (r c) -> p r c", r=RANK),
                    in_=hb[:, :].rearrange("r (p c) -> p r c", p=P),
                ).then_inc(sems["sdma"], 16)
                gp.wait_ge(sems["dset"], 2)
                gp.dma_start(
                    out=shift_vec[:].rearrange("(p c) -> p c", p=P),
                    in_=tv16[:],
                ).then_inc(sems["gset"], 16)
                # both DRAM vectors written before reading them back
                gp.wait_ge(sems["gset"], 32)
                # broadcast along partitions (stride-0 DRAM read)
                for vec, bc in ((scale_vec, scale_bc), (shift_vec, shift_bc)):
                    vv = vec[:]
                    gp.dma_start(
                        out=bc[:],
                        in_=bass.AP(tensor=vv.tensor, offset=vv.offset,
                                    ap=[[0, P]] + list(vv.ap)),
                    ).then_inc(sems["gset"], 16)

            @block.vector
            def _(v):
                v.memset(eps_t[:], EPS).then_inc(sems["const"], 1)
                # low-rank diagonals: diag = sum_r A[:,r]*B[r,:] * SCALING
                for (small, v16, k) in ((s_small, sv16, 1), (t_small, tv16, 2)):
                    v.wait_ge(sems["sdma"], 32 * k)
                    v.tensor_mul(
                        prod[:].rearrange("p (c r) -> p c r", c=C),
                        a_t[:].rearrange("p (c r) -> p c r", c=C),
                        b_t[:].rearrange("p (r c) -> p c r", r=RANK),
                    )
                    v.drain()
                    v.tensor_reduce(
                        out=small[:].rearrange("p (c u) -> p c u", u=1),
                        in_=prod[:].rearrange("p (c r) -> p c r", c=C),
                        axis=mybir.AxisListType.X,
                        op=mybir.AluOpType.add,
                    )
                    v.drain()
                    v.tensor_scalar_mul(small[:], small[:], SCALING)
                    v.drain()
                    v.tensor_copy(v16[:], small[:]).then_inc(sems["dset"], 1)

                def tt_pair(u):
                    # xh[u] = xh[u] * scale_bc + shift_bc  (f16, 2x rate)
                    if u == 0:
                        v.wait_ge(sems["gset"], 64)
                    v.wait_ge(sems["xh"], u + 1)
                    v.tensor_mul(xh[u % 2][:], xh[u % 2][:], scale_bc[:])
                    v.drain()
                    v.tensor_add(
                        xh[u % 2][:], xh[u % 2][:], shift_bc[:]
                    ).then_inc(sems["tt"], 1)

                for t in range(NTILES):
                    v.wait_ge(loadS[t % 2], 16 * (t // 2 + 1))
                    xt = xb[t % 2]
                    for c in range(NBN):
                        v.bn_stats(
                            out=stats[:].rearrange("p (c s) -> p c s", s=6)[
                                :, c, :
                            ],
                            in_=xt[:, c * BN_F:(c + 1) * BN_F],
                        )
                    v.drain()
                    v.bn_aggr(
                        out=mv[t % 2][:],
                        in_=stats[:].rearrange("p (c s) -> p c s", s=6),
                    ).then_inc(sems["stats"], 1)
                    # affine of the previous tile fills the sqrt round trip
                    if t >= 1:
                        tt_pair(t - 1)
                    v.wait_ge(sems["std"], t + 1)
                    v.reciprocal(rstdb[t % 2][:], stdb[t % 2][:])
                    v.drain()
                    v.scalar_tensor_tensor(
                        out=nmrb[t % 2][:],
                        in0=mv[t % 2][:, 0:1],
                        scalar=-1.0,
                        in1=rstdb[t % 2][:],
                        op0=MUL,
                        op1=MUL,
                    ).then_inc(sems["nmr"], 1)
                tt_pair(NTILES - 1)

            @block.scalar
            def _(sc):
                sc.wait_ge(sems["const"], 1)  # eps
                for t in range(NTILES):
                    sc.wait_ge(sems["stats"], t + 1)
                    sc.activation(
                        out=stdb[t % 2][:],
                        in_=mv[t % 2][:, 1:2],
                        func=mybir.ActivationFunctionType.Sqrt,
                        bias=eps_t[:],
                        scale=1.0,
                    ).then_inc(sems["std"], 1)
                    sc.wait_ge(sems["nmr"], t + 1)
                    if t >= 2:
                        # xh[t%2] free once store of tile t-2 done
                        sc.wait_ge(storeS[t % 2], 16 * (t // 2))
                    sc.activation(
                        out=xh[t % 2][:],
                        in_=xb[t % 2][:],
                        func=mybir.ActivationFunctionType.Identity,
                        bias=nmrb[t % 2][:],
                        scale=rstdb[t % 2][:],
                    ).then_inc(sems["xh"], 1)
                    sc.wait_ge(sems["tt"], t + 1)
                    sc.dma_start(
                        out=y[t * P:(t + 1) * P, :], in_=xh[t % 2][:]
                    ).then_inc(storeS[t % 2], 16)

    return nc


def kernel(x, lora_scale_A, lora_scale_B, lora_shift_A, lora_shift_B):
    x = np.ascontiguousarray(np.asarray(x, dtype=np.float32).reshape(-1, N))
    args = {
        "lora_scale_A": np.ascontiguousarray(lora_scale_A, dtype=np.float32),
        "lora_scale_B": np.ascontiguousarray(lora_scale_B, dtype=np.float32),
        "lora_shift_A": np.ascontiguousarray(lora_shift_A, dtype=np.float32),
        "lora_shift_B": np.ascontiguousarray(lora_shift_B, dtype=np.float32),
    }
    in_maps = [
        {"x_shard": x[i * ROWS:(i + 1) * ROWS], **args} for i in range(NCORES)
    ]
    nc = build_nc()
    res = run_bass_kernel_spmd(nc, in_maps, core_ids=list(range(NCORES)))
    out = np.concatenate(
        [np.asarray(res.results[i]["y_shard"]) for i in range(NCORES)], axis=0
    ).astype(np.float32)
    return out.reshape(B_DIM, S_DIM, N)


if __name__ == "__main__":
    import reference

    inputs = {k: np.asarray(v) for k, v in reference.setup_inputs().items()}
    expected = np.asarray(reference.reference(**inputs))
    actual = kernel(**inputs)
    err = np.abs(actual - expected)
    denom = np.abs(expected).max()
    print("max abs err:", err.max(), "rel:", err.max() / denom)
